# revision 96
# baseline (speedup 1.0000x reference)
"""Trainium2 Bass kernel for KeOps multi-head latent attention (v2).

Reference (B=2, N=2048, DIM=1024, LATENT=512, HEADS=16, HD=64):
    q = x @ wq * scale
    k = relu((x @ wkv[:, :D]) @ lk1) @ lk2   (folded: relu(x @ w1k) @ lk2)
    v = relu((x @ wkv[:, D:]) @ lv1) @ lv2
    per head: e = exp(q k^T + maskbias); out = (e @ v) / (e.sum + 1e-6)
    y = out @ wout + bout

Strategy (8 cores, one SPMD NEFF, NO collective):
  - queries: tokens sharded 512/core (cores 0-3 batch0, 4-7 batch1).
  - keys: masked keys compacted on host; EVERY core computes k/v for its
    batch's full active-key set (NB*128 slots) locally — redundant compute
    is far cheaper than the modeled AllGather (15us + 40GB/s).
  - all matmuls fp8e4m3 with DoubleRow (2x modeled PE throughput),
    including the output projection: v-path mean-centering makes att/wout
    fp8 affordable (the mean path rides the exact fp64 output bias).
  - v-path mean-centering: h_v has positive mean (relu); subtracting a
    host-computed statistical mean per latent (fp8-snapped) removes the
    coherent component of the lv2-fp8 quantization error; the mean path
    v0 = c @ lv2 rides through the output bias in fp64 (exact since
    softmax weights sum to 1). K-path coherent errors cancel in softmax.
  - scores per head via DoubleRow on 32-partition quadrants
    (tile_position): head dims split 32+32 across the two DR planes.
  - denominator: 64 'ones' columns interleaved with v give a PE-broadcast
    denominator on psum partitions 64:127 (free), so normalize is one
    reciprocal + one multiply on DVE per head.
  - exp split between ACT (true exp, fp8 out) and DVE (Schraudolph uint8
    bit-trick -> fp8e4m3, bit-exact validated on HW) to balance engines.
"""

import sys

sys.path.insert(0, "/opt/trn_rl_repo")
import numpy as np
import ml_dtypes
import concourse.bass as bass
import concourse.mybir as mybir
import concourse.tile as tile
from concourse import bacc
from concourse.bass_utils import run_bass_kernel_spmd

DIM, LATENT, HEADS, HD = 1024, 512, 16, 64
B, N, NC, T = 2, 2048, 8, 512
SCALE = HD ** -0.5
BF16, F32, FP8 = mybir.dt.bfloat16, mybir.dt.float32, mybir.dt.float8e4
U8 = mybir.dt.uint8
NPBF16 = ml_dtypes.bfloat16
NPFP8 = ml_dtypes.float8_e4m3
DR = mybir.MatmulPerfMode.DoubleRow

LN2 = float(np.log(2.0))
C_SHIFT = 5 * LN2            # exp shift; e^-C folded via bias, 2^-5 exact
NEGB = -35.0                 # pad-kill bias
C8 = 0.0435                  # schraudolph tuning constant
A8 = 8.0 / LN2
A8S = A8 * SCALE / 2.0       # DVE schr multiplier on raw scores
KB2R = 8.0 * (7.0 - C8) - A8 * C_SHIFT   # schr bias, real keys

_cache: dict = {}
LAST_RESULTS = None


def _build(NB, FPB):
    """NB = key blocks of 128 per batch; FPB = first block containing pads
    (blocks < FPB use constant exp bias; blocks >= FPB use per-slot AP)."""
    NK = NB * 128
    Exp = mybir.ActivationFunctionType.Exp
    Relu = mybir.ActivationFunctionType.Relu
    Copy = mybir.ActivationFunctionType.Copy
    Ident = mybir.ActivationFunctionType.Identity
    Alu = mybir.AluOpType

    nc = bacc.Bacc("TRN2", target_bir_lowering=False, num_devices=NC)
    xq_d = nc.dram_tensor("xq", [DIM, T], FP8, kind="ExternalInput")
    xkv_d = nc.dram_tensor("xkv", [DIM, NK], FP8, kind="ExternalInput")
    wq_d = nc.dram_tensor("wq", [DIM, DIM], FP8, kind="ExternalInput")
    w1k_d = nc.dram_tensor("w1k", [DIM, LATENT], FP8, kind="ExternalInput")
    w1v_d = nc.dram_tensor("w1v", [DIM, LATENT], FP8, kind="ExternalInput")
    lk2_d = nc.dram_tensor("lk2", [LATENT, DIM], FP8, kind="ExternalInput")
    lv2_d = nc.dram_tensor("lv2", [LATENT, DIM], FP8, kind="ExternalInput")
    wout_d = nc.dram_tensor("wout", [DIM, DIM], FP8, kind="ExternalInput")
    bout_d = nc.dram_tensor("bout2", [128, 8], F32, kind="ExternalInput")
    kbt_d = nc.dram_tensor("kbt", [128, NB], F32, kind="ExternalInput")
    kbt2_d = nc.dram_tensor("kbt2", [128, NB], F32, kind="ExternalInput")
    cv_d = nc.dram_tensor("cv", [128, 4], F32, kind="ExternalInput")
    y_d = nc.dram_tensor("yT", [DIM, T], BF16, kind="ExternalOutput")

    from contextlib import ExitStack
    with ExitStack() as ctx:
        tc = ctx.enter_context(tile.TileContext(nc))
        pool = lambda **kw: ctx.enter_context(tc.tile_pool(**kw))
        pw1 = pool(name="pw1", bufs=2)
        pl2 = pool(name="pl2", bufs=2)
        pwq = pool(name="pwq", bufs=1)
        pwo = pool(name="pwo", bufs=1)
        px = pool(name="px", bufs=1)
        ph = pool(name="ph", bufs=2)
        pkt = pool(name="pkt", bufs=1)
        pv = pool(name="pv", bufs=1)
        pqt = pool(name="pqt", bufs=1)
        patt = pool(name="patt", bufs=1)
        pe_ = pool(name="pe", bufs=8)
        pr = pool(name="pr", bufs=3)
        posb = pool(name="posb", bufs=5)
        psm = pool(name="psm", bufs=1)
        psA = pool(name="psA", bufs=3, space="PSUM")
        psN = pool(name="psN", bufs=2, space="PSUM")

        # ---------------- input DMAs ------------------------------------
        # split the critical first loads in halves so h matmuls start early
        w1k_sb = pw1.tile([128, 8 * LATENT], FP8, tag="w1")
        w1v_sb = pw1.tile([128, 8 * LATENT], FP8, tag="w1")
        xkv_sb = px.tile([128, 8 * NK], FP8, tag="xkv")
        cvt = psm.tile([128, 4], F32, tag="cv")
        xkv3v = xkv_sb[:].rearrange("p (d n) -> p d n", d=8)
        w1k3v = w1k_sb[:].rearrange("p (d l) -> p d l", d=8)
        w1v3v = w1v_sb[:].rearrange("p (d l) -> p d l", d=8)
        for hf in range(2):
            dd = slice(512 * hf, 512 * (hf + 1))
            nc.sync.dma_start(
                xkv3v[:, 4 * hf:4 * hf + 4],
                xkv_d.ap()[dd, :].rearrange("(d p) n -> p d n", p=128))
            nc.sync.dma_start(
                w1k3v[:, 4 * hf:4 * hf + 4],
                w1k_d.ap()[dd, :].rearrange("(d p) l -> p d l", p=128))
            nc.sync.dma_start(
                w1v3v[:, 4 * hf:4 * hf + 4],
                w1v_d.ap()[dd, :].rearrange("(d p) l -> p d l", p=128))
        nc.sync.dma_start(cvt[:], cv_d.ap())

        lk2_sb = pl2.tile([128, 4 * DIM], FP8, tag="l2")
        lv2_sb = pl2.tile([128, 4 * DIM], FP8, tag="l2")
        nc.sync.dma_start(
            lk2_sb[:].rearrange("p (l c) -> p l c", l=4),
            lk2_d.ap().rearrange("(l p) c -> p l c", p=128))
        nc.sync.dma_start(
            lv2_sb[:].rearrange("p (l c) -> p l c", l=4),
            lv2_d.ap().rearrange("(l p) c -> p l c", p=128))

        wq_sb = pwq.tile([128, 8 * DIM], FP8, tag="wq")
        xq_sb = px.tile([128, 8 * T], FP8, tag="xq")
        nc.sync.dma_start(
            wq_sb[:].rearrange("p (d c) -> p d c", d=8),
            wq_d.ap().rearrange("(d p) c -> p d c", p=128))
        nc.sync.dma_start(
            xq_sb[:].rearrange("p (d n) -> p d n", d=8),
            xq_d.ap().rearrange("(d p) n -> p d n", p=128))

        kbt = psm.tile([128, NB], F32, tag="kbt")
        kbt2 = psm.tile([128, NB], F32, tag="kbt2")
        nc.sync.dma_start(kbt[:], kbt_d.ap())
        nc.sync.dma_start(kbt2[:], kbt2_d.ap())

        wout_sb = pwo.tile([128, 8 * DIM], FP8, tag="wo")
        boutt = psm.tile([128, 8], F32, tag="bo")
        nc.sync.dma_start(
            wout_sb[:].rearrange("p (d c) -> p d c", d=8),
            wout_d.ap().rearrange("(d p) c -> p d c", p=128))
        nc.sync.dma_start(boutt[:], bout_d.ap())

        # 3D chunk-major views
        w1k3 = w1k_sb[:].rearrange("p (d l) -> p d l", d=8)
        w1v3 = w1v_sb[:].rearrange("p (d l) -> p d l", d=8)
        xkv3 = xkv_sb[:].rearrange("p (d n) -> p d n", d=8)
        lk23 = lk2_sb[:].rearrange("p (l c) -> p l c", l=4)
        lv23 = lv2_sb[:].rearrange("p (l c) -> p l c", l=4)
        wq3 = wq_sb[:].rearrange("p (d c) -> p d c", d=8)
        xq3 = xq_sb[:].rearrange("p (d n) -> p d n", d=8)

        hk_sb = ph.tile([128, 4 * NK], FP8, tag="h")
        hv_sb = ph.tile([128, 4 * NK], FP8, tag="h")
        hk3 = hk_sb[:].rearrange("p (l n) -> p l n", l=4)
        hv3 = hv_sb[:].rearrange("p (l n) -> p l n", l=4)
        kt_sb = pkt.tile([128, 8 * NK], FP8, tag="kt")
        v_sb = pv.tile([128, NB * 2048], FP8, tag="v")
        qt_sb = pqt.tile([128, 8 * T], FP8, tag="qt")
        att_sb = patt.tile([128, 8 * T], FP8, tag="att")

        KEY_CH = [(0, 1024)] + ([(1024, NK - 1024)] if NK > 1024 else [])

        # ones columns for the PE-broadcast denominator (Pool, idle engine)
        v4 = v_sb[:].rearrange("p (j h two d) -> p (j h) two d",
                               j=NB, h=HEADS, two=2)
        nc.gpsimd.memset(v4[:, :, 1, :], 1.0)

        def h_path_l(w13, dst3, is_v, l):
            if True:
                ps = psA.tile([128, 1024], F32, tag="big")
                pst = psN.tile([128, 512], F32, tag="nm")
                for g0 in (0, 512):
                    for dp in range(4):
                        nc.tensor.matmul(
                            ps[:, g0:g0 + 512],
                            w13[:, 2 * dp:2 * dp + 2, 128 * l:128 * (l + 1)],
                            xkv3[:, 2 * dp:2 * dp + 2, g0:g0 + 512],
                            start=(dp == 0), stop=(dp == 3), perf_mode=DR)
                if NK > 1024:
                    for dp in range(4):
                        nc.tensor.matmul(
                            pst[:, 0:NK - 1024],
                            w13[:, 2 * dp:2 * dp + 2, 128 * l:128 * (l + 1)],
                            xkv3[:, 2 * dp:2 * dp + 2, 1024:NK],
                            start=(dp == 0), stop=(dp == 3), perf_mode=DR)
                with nc.allow_low_precision(reason="fp8 latents"):
                    if is_v:
                        # (max(ps,0) - cv) on DVE; centering the v-latents
                        nc.vector.tensor_scalar(
                            dst3[:, l, 0:1024], ps[:], 0.0, cvt[:, l:l + 1],
                            Alu.max, Alu.subtract)
                        if NK > 1024:
                            nc.vector.tensor_scalar(
                                dst3[:, l, 1024:NK], pst[:, 0:NK - 1024],
                                0.0, cvt[:, l:l + 1], Alu.max, Alu.subtract)
                    else:
                        nc.scalar.activation(dst3[:, l, 0:1024], ps[:], Relu,
                                             scale=2.0 ** -5)
                        if NK > 1024:
                            nc.scalar.activation(dst3[:, l, 1024:NK],
                                                 pst[:, 0:NK - 1024], Relu,
                                                 scale=2.0 ** -5)

        for l in range(4):
            h_path_l(w1k3, hk3, False, l)
            h_path_l(w1v3, hv3, True, l)

        # ---------------- kT (8 chunks), v (NB blocks), q (4 pairs) ------
        def kt_chunk(c8):
            ps = psA.tile([128, 1024], F32, tag="big")
            pst = psN.tile([128, 512], F32, tag="nm")
            for g0 in (0, 512):
                for lp in range(2):
                    nc.tensor.matmul(
                        ps[:, g0:g0 + 512],
                        lk23[:, 2 * lp:2 * lp + 2, 128 * c8:128 * (c8 + 1)],
                        hk3[:, 2 * lp:2 * lp + 2, g0:g0 + 512],
                        start=(lp == 0), stop=(lp == 1), perf_mode=DR)
            if NK > 1024:
                for lp in range(2):
                    nc.tensor.matmul(
                        pst[:, 0:NK - 1024],
                        lk23[:, 2 * lp:2 * lp + 2, 128 * c8:128 * (c8 + 1)],
                        hk3[:, 2 * lp:2 * lp + 2, 1024:NK],
                        start=(lp == 0), stop=(lp == 1), perf_mode=DR)
            # kT evac on DVE (kv-phase engine balance: ACT has relu+v+q)
            with nc.allow_low_precision(reason="fp8 k"):
                nc.vector.tensor_scalar(
                    kt_sb[:, c8 * NK:c8 * NK + 1024], ps[:],
                    2.0 ** -4, None, Alu.mult)
                if NK > 1024:
                    nc.vector.tensor_scalar(
                        kt_sb[:, c8 * NK + 1024:(c8 + 1) * NK],
                        pst[:, 0:NK - 1024], 2.0 ** -4, None, Alu.mult)

        def v_block(j):
            ps = psA.tile([128, 1024], F32, tag="big")
            for ch in range(2):
                for lp in range(2):
                    nc.tensor.matmul(
                        ps[:, 512 * ch:512 * (ch + 1)],
                        hv3[:, 2 * lp:2 * lp + 2, 128 * j:128 * (j + 1)],
                        lv23[:, 2 * lp:2 * lp + 2, 512 * ch:512 * (ch + 1)],
                        start=(lp == 0), stop=(lp == 1), perf_mode=DR)
            dst = v_sb[:, j * 2048:(j + 1) * 2048] \
                .rearrange("p (h two d) -> p h two d", h=16, two=2)[:, :, 0, :]
            with nc.allow_low_precision(reason="fp8 v"):
                if j % 3 == 2:
                    nc.vector.tensor_scalar(
                        dst, ps[:].rearrange("p (h d) -> p h d", h=16),
                        2.0 ** -8, None, Alu.mult)
                else:
                    nc.scalar.activation(
                        dst, ps[:].rearrange("p (h d) -> p h d", h=16),
                        Copy, scale=2.0 ** -8)

        def q_pair(t):
            ps = psA.tile([128, 1024], F32, tag="big")
            for pl in range(2):
                for dp in range(4):
                    nc.tensor.matmul(
                        ps[:, 512 * pl:512 * (pl + 1)],
                        wq3[:, 2 * dp:2 * dp + 2,
                            (2 * t + pl) * 128:(2 * t + pl + 1) * 128],
                        xq3[:, 2 * dp:2 * dp + 2, :],
                        start=(dp == 0), stop=(dp == 3), perf_mode=DR)
            with nc.allow_low_precision(reason="fp8 q"):
                nc.scalar.activation(qt_sb[:, t * 1024:(t + 1) * 1024], ps[:],
                                     Copy)

        order = [("k", 0), ("q", 0), ("k", 1), ("v", 0), ("q", 1),
                 ("k", 2), ("v", 1), ("k", 3), ("v", 2), ("q", 2),
                 ("k", 4), ("v", 3), ("k", 5), ("v", 4), ("q", 3),
                 ("k", 6), ("v", 5), ("k", 7), ("v", 6), ("v", 7), ("v", 8)]
        order = [(k, i) for (k, i) in order if
                 (k != "v" or i < NB) and (k != "k" or i < 8)]
        for kind, i in order:
            (kt_chunk if kind == "k" else v_block if kind == "v" else q_pair)(i)

        def drain_kv(n):
            pass

        # ---------------- attention -------------------------------------
        # flat software pipeline across all (head, unit) pairs: scores run
        # two units ahead of exp; numer three behind; each head's normalize
        # is deferred until after the NEXT head's DVE exps so the
        # exp->numer->recip round trip never stalls DVE.  exp split: ACT
        # gets pad-free pairs 0,1,2 (const bias); DVE gets pair 3 + the
        # leftover block + reciprocal + normalize multiply.
        NPAIR = NB // 2
        NU = NPAIR + (1 if NB % 2 else 0)
        v3 = v_sb[:].rearrange("p (j x) -> p j x", j=NB)

        scs = {}
        es = {}
        nms = {}
        pending_norm = []

        def head_views(h):
            t, g = h // 4, h % 4
            kt3 = kt_sb[:, t * 2 * NK:(t + 1) * 2 * NK] \
                .rearrange("p (pl k) -> p pl k", pl=2)
            qt3 = qt_sb[:, t * 1024:(t + 1) * 1024] \
                .rearrange("p (pl n) -> p pl n", pl=2)
            return kt3, qt3[32 * g:32 * (g + 1), :, :], 32 * g

        def emit_sc(h, u):
            kt3, lhq, bp32 = head_views(h)
            if u < NPAIR:
                sc = psA.tile([128, 1024], F32, tag="big", name=f"sc{h}_{u}")
            else:
                # leftover unit lives in the psN ring so the psA ring
                # advances by 4 (not 5) per head: the next head's first sc
                # then reuses a slot freed by an early DVE exp, not ACT's
                # last one
                sc = psN.tile([128, 512], F32, tag="nm", name=f"sc{h}_{u}")
            scs[(h, u)] = sc
            nj = 2 if u < NPAIR else 1
            for half in range(nj):
                j = 2 * u + half
                nc.tensor.matmul(
                    sc[:, 512 * half:512 * (half + 1)],
                    kt3[bp32:bp32 + 32, :, 128 * j:128 * (j + 1)],
                    lhq, start=True, stop=True, perf_mode=DR,
                    tile_position=(bp32, 0))

        def emit_exp(h, u):
            sc = scs[(h, u)]
            if u < NPAIR:
                e = pe_.tile([128, 1024], FP8, tag="e")
                with nc.allow_low_precision(reason="fp8 softmax"):
                    if 2 * u + 1 < FPB and u < 3:
                        nc.scalar.activation(e[:], sc[:], Exp,
                                             bias=kbt[:, 0:1],
                                             scale=SCALE / 2)
                    else:
                        for half in range(2):
                            j = 2 * u + half
                            nc.vector.tensor_scalar(
                                e[:, 512 * half:512 * (half + 1)].bitcast(U8),
                                sc[:, 512 * half:512 * (half + 1)], A8S,
                                KB2R if j < FPB else kbt2[:, j:j + 1],
                                Alu.mult, Alu.add)
            else:
                j = 2 * NPAIR
                e = pe_.tile([128, 512], FP8, tag="e1")
                with nc.allow_low_precision(reason="fp8 softmax"):
                    if h % 2 == 1:
                        nc.scalar.activation(e[:], sc[:], Exp,
                                             bias=kbt[:, j:j + 1],
                                             scale=SCALE / 2)
                    else:
                        nc.vector.tensor_scalar(
                            e[:].bitcast(U8), sc[:], A8S,
                            KB2R if j < FPB else kbt2[:, j:j + 1],
                            Alu.mult, Alu.add)
            es[(h, u)] = e
            # emit the deferred normalize after this head's first DVE exp
            if u == 3 and pending_norm:
                pending_norm.pop(0)()

        def emit_nm(h, u):
            if u == 0:
                nms[h] = psN.tile([128, 512], F32, tag="nm", name=f"nm{h}")
            nm = nms[h]
            if u < NPAIR:
                nc.tensor.matmul(
                    nm[:], v3[:, 2 * u:2 * u + 2, 128 * h:128 * (h + 1)],
                    es[(h, u)][:].rearrange("p (two n) -> p two n", two=2),
                    start=(u == 0), stop=(u == NU - 1), perf_mode=DR)
            else:
                nc.tensor.matmul(
                    nm[:], v3[:, 2 * NPAIR, 128 * h:128 * (h + 1)],
                    es[(h, u)][:], start=False, stop=True,
                    skip_group_check=True)
            if u == NU - 1:
                def normalize(h=h, nm=nm):
                    rr = pr.tile([64, 512], F32, tag="r")
                    nc.vector.reciprocal(rr[:], nm[64:128, :])
                    with nc.allow_low_precision(reason="bf16 att"):
                        nc.vector.tensor_mul(
                            att_sb[64 * (h % 2):64 * (h % 2) + 64,
                                   (h // 2) * T:(h // 2 + 1) * T],
                            nm[0:64, :], rr[:])
                pending_norm.append(normalize)

        units = [(h, u) for h in range(HEADS) for u in range(NU)]
        LA_E, LA_N = 1, 5
        for i in range(len(units) + LA_N):
            if i < len(units):
                emit_sc(*units[i])
            if LA_E <= i and i - LA_E < len(units):
                emit_exp(*units[i - LA_E])
            drain_kv(1)
            if LA_N <= i and i - LA_N < len(units):
                emit_nm(*units[i - LA_N])
        while pending_norm:
            pending_norm.pop(0)()

        # ---------------- output projection ------------------------------
        wo3 = wout_sb[:].rearrange("p (d c) -> p d c", d=8)
        att3 = att_sb[:].rearrange("p (c n) -> p c n", c=8)
        for cb in range(8):
            ps = psA.tile([128, 1024], F32, tag="big")
            for c2 in range(4):
                nc.tensor.matmul(
                    ps[:, 0:512],
                    wo3[:, 2 * c2:2 * c2 + 2, 128 * cb:128 * (cb + 1)],
                    att3[:, 2 * c2:2 * c2 + 2, :],
                    start=(c2 == 0), stop=(c2 == 3), perf_mode=DR)
            osb = posb.tile([128, 512], BF16, tag="osb")
            with nc.allow_low_precision(reason="bf16 output"):
                nc.vector.tensor_scalar(osb[:], ps[:, 0:512],
                                        2.0 ** -7, boutt[:, cb:cb + 1],
                                        Alu.mult, Alu.add)
            nc.sync.dma_start(y_d.ap()[128 * cb:128 * (cb + 1), :], osb[:])

    nc.compile()
    return nc


def _f8(x):
    return np.asarray(x, np.float32).astype(NPFP8)


def kernel(x, mask, wq, wkv, lk1, lk2, lv1, lv2, wout, bout, **kw):
    global LAST_RESULTS
    x = np.asarray(x, np.float32)
    mask = np.asarray(mask)
    wq = np.asarray(wq, np.float64)
    wkv = np.asarray(wkv, np.float64)
    lk1 = np.asarray(lk1, np.float64)
    lk2 = np.asarray(lk2, np.float64)
    lv1 = np.asarray(lv1, np.float64)
    lv2 = np.asarray(lv2, np.float64)
    wout = np.asarray(wout, np.float64)
    bout = np.asarray(bout, np.float64)

    act = [np.nonzero(np.asarray(mask[b]) == 1)[0] for b in range(B)]
    A = [len(a) for a in act]
    NB = max(1, (max(A) + 127) // 128)
    NK = NB * 128
    FPB = min(A) // 128          # first block that contains pad slots

    # column permutation for the DR-32 scores layout:
    # psum chunk (t,pl) partitions = [head 4t+g, dims 32pl..32pl+32]
    perm = np.array([64 * (4 * t + g) + 32 * pl + i
                     for t in range(4) for pl in range(2)
                     for g in range(4) for i in range(32)])

    w1k = wkv[:, :DIM] @ lk1
    w1v = wkv[:, DIM:] @ lv1
    w1k8 = _f8(32 * w1k)
    w1v8 = _f8(32 * w1v)
    lk28 = _f8(32 * lk2[:, perm])
    lv28 = _f8(32 * lv2)
    wq8 = _f8(wq[:, perm])
    woutb = _f8(32.0 * wout)

    # v-path centering: statistical mean of relu(w1v8 . x) per latent,
    # snapped to the fp8 grid so exact relu-zeros quantize exactly
    xr = float(np.sqrt((x.astype(np.float64) ** 2).mean()))
    colv = np.sqrt((w1v8.astype(np.float64) ** 2).sum(0)) * xr
    cv32 = _f8(0.39894228 * colv).astype(np.float64)       # scale-32 units
    v0 = (cv32 / 32.0) @ lv2                               # exact lv2
    bout2 = bout + v0 @ wout
    bout2_t = np.ascontiguousarray(
        bout2.reshape(8, 128).T.astype(np.float32))
    cv_t = np.ascontiguousarray(
        cv32.reshape(4, 128).T.astype(np.float32))

    # exp biases per key slot (per batch)
    kbt = np.full((B, NK), NEGB, np.float32)
    kbt2 = np.full((B, NK), KB2R + A8 * (NEGB + C_SHIFT), np.float32)
    for b in range(B):
        kbt[b, :A[b]] = -C_SHIFT
        kbt2[b, :A[b]] = KB2R
    kbt_t = [np.ascontiguousarray(kbt[b].reshape(NB, 128).T) for b in range(B)]
    kbt2_t = [np.ascontiguousarray(kbt2[b].reshape(NB, 128).T) for b in range(B)]

    key = (NB, FPB)
    if key not in _cache:
        _cache[key] = _build(NB, FPB)
    nc = _cache[key]

    x_flat = x.reshape(B * N, DIM)
    xkv_b = []
    for b in range(B):
        xkv = np.zeros((DIM, NK), NPFP8)
        xkv[:, :A[b]] = _f8(x_flat[b * N + act[b]].T)
        xkv_b.append(xkv)

    in_maps = []
    for c in range(NC):
        b = c // 4
        in_maps.append({
            "xq": np.ascontiguousarray(_f8(x_flat[c * T:(c + 1) * T].T)),
            "xkv": xkv_b[b],
            "wq": wq8, "w1k": w1k8, "w1v": w1v8, "lk2": lk28, "lv2": lv28,
            "wout": woutb, "bout2": bout2_t, "kbt": kbt_t[b],
            "kbt2": kbt2_t[b], "cv": cv_t,
        })

    res = run_bass_kernel_spmd(nc, in_maps, core_ids=list(range(NC)))
    LAST_RESULTS = res
    y = np.empty((B * N, DIM), np.float32)
    for c in range(NC):
        y[c * T:(c + 1) * T] = res.results[c]["yT"].T.astype(np.float32)
    return y.reshape(B, N, DIM)
